# revision 4
# baseline (speedup 1.0000x reference)
"""Trainium2 Bass kernel for hierarchical softmax tree posterior (HNet.predict).

v5: fp16 row-pair (G=2) tree with all DVE ops <= 2 free dims and all
sigma ACT writes as 1-free-dim stride-2 fp16 (the HW-microbenchmarked
fast patterns).  See kernel2/kernel3 history: 3-free-dim DVE APs and
stride-8 / multi-dim strided fp16 ACT writes fall off the RTL fast path.

Layout: all post-PSUM tensors [128, node, 2] fp16, the pair dim packing
two batch row-tiles (bp*256+e*128+p).  Output fp16 (leaf, e)-packed,
host de-interleaves + upcasts.
"""

import contextlib

import numpy as np

import concourse.bacc as bacc
import concourse.mybir as mybir
import concourse.tile as tile
from concourse.bass_utils import run_bass_kernel_spmd

B, D = 8192, 64
NODES = 4095
LEAVES = 4096
NCORES = 8
BLOC = B // NCORES
KA = D + 1
NBP = 4               # row-pair groups of 256 rows

F32 = mybir.dt.float32
F16 = mybir.dt.float16
MM_DT = mybir.dt.float32r


def _build(reps=1, do_compile=True):
    nc = bacc.Bacc("TRN2", target_bir_lowering=False, debug=False, num_devices=NCORES)
    wdt = nc.dram_tensor("wdt", [KA, LEAVES], MM_DT, kind="ExternalInput")
    xt = nc.dram_tensor("xt", [KA, BLOC], MM_DT, kind="ExternalInput")
    out = nc.dram_tensor("out", [NBP * 128, LEAVES * 2], F16, kind="ExternalOutput")

    SIG = mybir.ActivationFunctionType.Sigmoid
    IDN = mybir.ActivationFunctionType.Identity

    with tile.TileContext(nc) as tc:
        with (
            tc.tile_pool(name="const", bufs=1) as const,
            tc.tile_pool(name="pa", bufs=1) as pa,
            tc.tile_pool(name="pb", bufs=2) as pb,
            tc.tile_pool(name="ps", bufs=2, space="PSUM") as psp,
        ):
            wdt_r = const.tile([KA, LEAVES], MM_DT)
            xt_r = const.tile([KA, BLOC], MM_DT)
            nc.sync.dma_start(out=wdt_r[:], in_=wdt[:])
            nc.sync.dma_start(out=xt_r[:], in_=xt[:])

            loop = tc.For_i(0, reps, 1) if reps > 1 else contextlib.nullcontext()
            with loop:
                _emit_body(nc, tc, pa, pb, psp, wdt_r, xt_r, out, SIG, IDN)

    if do_compile:
        nc.compile()
    return nc


def _emit_body(nc, tc, pa, pb, psp, wdt_r, xt_r, out, SIG, IDN):
    # per-bp sigma tiles for levels 0..9 (nodes 0..1022, col 1023 junk):
    # ACT writes are [node, stride 2] 1-free-dim.
    s_small = [pa.tile([128, 1024, 2], F16, tag=f"s_small{bp}", name=f"s_small{bp}")
               for bp in range(NBP)]
    pA = [pa.tile([128, 512, 2], F16, tag=f"pA{bp}", name=f"pA{bp}")
          for bp in range(NBP)]
    pB = [pa.tile([128, 512, 2], F16, tag=f"pB{bp}", name=f"pB{bp}")
          for bp in range(NBP)]
    p10 = [pa.tile([128, 1024, 2], F16, tag=f"p10_{bp}", name=f"p10_{bp}")
           for bp in range(NBP)]

    # ---- phase A: matmul + sigmoid for nodes 0..1023, per (bp, e)
    for bp in range(NBP):
        for e in range(2):
            bt = bp * 2 + e
            ps = psp.tile([128, 1024], F32, tag="ps")
            for c in range(2):
                nc.tensor.matmul(
                    ps[:, c * 512:(c + 1) * 512],
                    xt_r[:, bt * 128:(bt + 1) * 128],
                    wdt_r[:, c * 512:(c + 1) * 512],
                    start=True, stop=True,
                )
            nc.scalar.activation(out=s_small[bp][:, :, e], in_=ps[:], func=SIG)

    # ---- tree levels 0..9 per bp (all ops 2 free dims, inner e-pairs)
    for bp in range(NBP):
        nc.vector.tensor_copy(pA[bp][:, 0:1, :], s_small[bp][:, 0:1, :])
        nc.scalar.activation(out=pA[bp][:, 1:2, :], in_=s_small[bp][:, 0:1, :],
                             func=IDN, bias=1.0, scale=-1.0)
    for lvl in range(1, 10):
        n = 1 << lvl
        off = n - 1
        for bp in range(NBP):
            cur = pA[bp] if lvl % 2 == 1 else pB[bp]
            nxt = p10[bp] if lvl == 9 else (pB[bp] if lvl % 2 == 1 else pA[bp])
            nxt4 = nxt[:, 0:2 * n, :].rearrange("p (m two) e -> p m two e", two=2)
            nc.vector.tensor_mul(nxt4[:, :, 0, :], cur[:, 0:n, :],
                                 s_small[bp][:, off:off + n, :])
            nc.vector.tensor_sub(nxt4[:, :, 1, :], cur[:, 0:n, :],
                                 nxt4[:, :, 0, :])

    # ---- phase B per bp: levels 10..11 (same as kernel2; all fast APs)
    for bp in range(NBP):
        s10 = pb.tile([128, 1024, 2], F16, tag="s10")
        s11 = pb.tile([128, 2048, 2], F16, tag="s11")
        for e in range(2):
            bt = bp * 2 + e
            ps1 = psp.tile([128, 1024], F32, tag="ps")
            for c in range(2):
                nc.tensor.matmul(
                    ps1[:, c * 512:(c + 1) * 512],
                    xt_r[:, bt * 128:(bt + 1) * 128],
                    wdt_r[:, 1023 + c * 512:1023 + (c + 1) * 512],
                    start=True, stop=True,
                )
            ps2 = psp.tile([128, 2048], F32, tag="ps")
            for c in range(4):
                nc.tensor.matmul(
                    ps2[:, c * 512:(c + 1) * 512],
                    xt_r[:, bt * 128:(bt + 1) * 128],
                    wdt_r[:, 2047 + c * 512:2047 + (c + 1) * 512],
                    start=True, stop=True,
                )
            nc.scalar.activation(out=s10[:, :, e], in_=ps1[:], func=SIG)
            nc.scalar.activation(out=s11[:, :, e], in_=ps2[:], func=SIG)

        p11 = pb.tile([128, 2048, 2], F16, tag="p11")
        p11v = p11.rearrange("p (m two) e -> p m two e", two=2)
        nc.vector.tensor_mul(p11v[:, :, 0, :], p10[bp][:], s10[:])
        nc.vector.tensor_sub(p11v[:, :, 1, :], p10[bp][:], p11v[:, :, 0, :])

        ot = pb.tile([128, 4096, 2], F16, tag="ot")
        otv = ot.rearrange("p (m two) e -> p m two e", two=2)
        nc.vector.tensor_mul(otv[:, :, 0, :], p11[:], s11[:])
        nc.vector.tensor_sub(otv[:, :, 1, :], p11[:], otv[:, :, 0, :])

        nc.sync.dma_start(
            out=out[bp * 128:(bp + 1) * 128, :],
            in_=ot.rearrange("p m e -> p (m e)"),
        )


_NC_CACHE = {}


def _get_nc(reps=1):
    if reps not in _NC_CACHE:
        _NC_CACHE[reps] = _build(reps)
    return _NC_CACHE[reps]


def _prep_inputs(x, W, b):
    x = np.asarray(x, dtype=np.float32)
    W = np.asarray(W, dtype=np.float32)
    b = np.asarray(b, dtype=np.float32)
    Wd = W[:, 0, :] - W[:, 1, :]
    bd = b[:, 0] - b[:, 1]
    wdt = np.zeros((KA, LEAVES), dtype=np.float32)
    wdt[:D, :NODES] = Wd.T
    wdt[D, :NODES] = bd
    xt = np.empty((KA, B), dtype=np.float32)
    xt[:D] = x.T
    xt[D] = 1.0
    return [
        {"wdt": wdt, "xt": np.ascontiguousarray(xt[:, c * BLOC:(c + 1) * BLOC])}
        for c in range(NCORES)
    ]


def _unpack_out(res):
    parts = []
    for c in range(NCORES):
        a = res.results[c]["out"].reshape(NBP, 128, LEAVES, 2)
        a = a.transpose(0, 3, 1, 2).reshape(BLOC, LEAVES)
        parts.append(a)
    return np.concatenate(parts, axis=0).astype(np.float32)


def kernel(x, W, b):
    in_maps = _prep_inputs(x, W, b)
    nc = _get_nc()
    res = run_bass_kernel_spmd(nc, in_maps, core_ids=list(range(NCORES)))
    return _unpack_out(res)


if __name__ == "__main__":
    rng = np.random.default_rng(0)
    x = rng.standard_normal((B, D)).astype(np.float32)
    W = (rng.standard_normal((NODES, 2, D)) * 0.1).astype(np.float32)
    b = (rng.standard_normal((NODES, 2)) * 0.1).astype(np.float32)
    p = kernel(x, W, b)
    print("out", p.shape, p.dtype, "rowsum", p.sum(axis=1)[:4])
